# revision 69
# baseline (speedup 1.0000x reference)
"""Multi-head attention (B=2, S=2048, H=1024, 16 heads) on 8 TRN2 NeuronCores.

Sharding: core c -> batch b = c//4, head-group g = c%4 (heads 4g..4g+3).
Each core computes q/k/v projections for its 4 heads (tensor parallel),
full attention for those heads, and a partial output projection
(contribution of its 256 hidden dims). Host sums the 4 partials per batch
and adds the output bias.

v2 schedule: the ACT engine (128 exps of [128,1024], ~1.1us each) is the
hard floor (~142us); PE work (~640 x 512-col passes ~ 137us) is list-
scheduled into a single budget-paced stream so the PE never idles (idle
drops the PE p-state to 1.2GHz for ~3us) and every scores tile is ready
ahead of its exp.

Key kernel tricks vs the naive layout:
  - scores: two K=64 matmuls row-packed at partition base 0/64 run
    concurrently (both heads of a pair in one 512-col pass).
  - PV stationary per head is [v_h | ones*64] ([1s*64 | v_h] for the odd
    head): M=128 costs the same N cycles as M=65, and the softmax row-sum
    l comes out of PSUM already replicated across 64 partitions.
  - epilogue per (pair, ib): DVE reciprocal_approx_fast on the replicated
    l rows (PSUM in, partition-shifted out), then two DVE multiplies
    pv*rl -> ctxn bf16. No ACT work, no broadcast matmuls, no shift DMAs.
  - PE warmup matmuls + a dummy exp during the initial DMA wait (p-state
    ramp + act-table load off the critical path).
  - DMA dispatch split across the two hardware DGE queues (sync + ACT).
"""

import os
from contextlib import ExitStack

import numpy as np
import ml_dtypes

B = 2
S = 2048
HID = 1024
NHEAD = 16
HDIM = 64
NCORES = 8
GROUPS = 4  # head-groups per batch (cores per batch)
DH = 256  # hidden dims per core (4 heads x 64)
SCALE = 1.0 / np.sqrt(np.float32(HDIM))  # 0.125

NSTEPS = 128  # 8 blocks x 16 j-chunks
JPB = 16

# scheduler tunables (us of PE work per atom; X = ACT pace per step)
COST_SCORE = 0.225
COST_PV = 0.43
COST_QK = 0.215
COST_V = 0.115
COST_OP = 0.43
X_PACE = 1.10
MIN_STEP = 0.75
MINAGE_B0 = 8
MINAGE = 2
MAXLAG = 22
NEX = 26
PV_PAUSE = 1  # steps to pause PV pops after a block's last PV (pv bank WAR)
N_WARM = 12

_CACHE = {}
last_exec_time_ns = None
last_results = None


def _build_graph(with_qkv_bias: bool):
    import concourse.bass as bass
    import concourse.mybir as mybir
    import concourse.tile as tile
    from concourse import bacc

    F32 = mybir.dt.float32
    BF16 = mybir.dt.bfloat16
    EXP = mybir.ActivationFunctionType.Exp

    nc = bacc.Bacc()
    xt_d = nc.declare_dram_parameter("xt", [HID, S], BF16, isOutput=False)
    wq_d = nc.declare_dram_parameter("wq", [HID, DH], BF16, isOutput=False)
    wk_d = nc.declare_dram_parameter("wk", [HID, DH], BF16, isOutput=False)
    wv_d = nc.declare_dram_parameter("wv", [HID, DH], BF16, isOutput=False)
    wo_d = nc.declare_dram_parameter("wo", [DH, HID], BF16, isOutput=False)
    if with_qkv_bias:
        bq_d = nc.declare_dram_parameter("bq", [1, DH], BF16, isOutput=False)
        bk_d = nc.declare_dram_parameter("bk", [1, DH], BF16, isOutput=False)
        bv_d = nc.declare_dram_parameter("bv", [1, DH], BF16, isOutput=False)
    out_d = nc.declare_dram_parameter("out", [S, HID], F32, isOutput=True)

    with ExitStack() as ctx:
        tc = ctx.enter_context(tile.TileContext(nc))
        cons = ctx.enter_context(tc.tile_pool(name="cons", bufs=1))
        work = ctx.enter_context(tc.tile_pool(name="work", bufs=3))
        exq = ctx.enter_context(tc.tile_pool(name="exq", bufs=NEX))
        scp = ctx.enter_context(tc.tile_pool(name="scp", bufs=2, space="PSUM"))
        pvp = ctx.enter_context(tc.tile_pool(name="pvp", bufs=1, space="PSUM"))
        mip = ctx.enter_context(tc.tile_pool(name="mip", bufs=2, space="PSUM"))

        # ---- persistent tiles -------------------------------------------
        xt_sb = [
            cons.tile([128, S], BF16, name=f"xt{e}", tag=f"xt{e}") for e in range(8)
        ]
        wq_sb = [
            cons.tile([128, DH], BF16, name=f"wq{e}", tag=f"wq{e}") for e in range(8)
        ]
        wk_sb = [
            cons.tile([128, DH], BF16, name=f"wk{e}", tag=f"wk{e}") for e in range(8)
        ]
        wv_sb = [
            cons.tile([128, DH], BF16, name=f"wv{e}", tag=f"wv{e}") for e in range(8)
        ]
        wo_sb = [
            cons.tile([128, HID], BF16, name=f"wo{e}", tag=f"wo{e}") for e in range(2)
        ]
        qt_sb = [
            cons.tile([128, S], BF16, name=f"qt{c}", tag=f"qt{c}") for c in range(2)
        ]
        kt_sb = [
            cons.tile([128, S], BF16, name=f"kt{c}", tag=f"kt{c}") for c in range(2)
        ]
        # v per j-chunk: two 256-col pair blocks [v_even | ones128 | v_odd]
        v_sb = [
            cons.tile([128, 512], BF16, name=f"v{j}", tag=f"v{j}") for j in range(16)
        ]
        ctxn_sb = [
            [
                cons.tile([128, 512], BF16, name=f"cx{c}_{i}", tag=f"cx{c}_{i}")
                for i in range(4)
            ]
            for c in range(2)
        ]

        # ---- memsets (engines idle during DMA wait) ---------------------
        warm = cons.tile([128, 512], BF16, name="warm", tag="warm")
        nc.vector.memset(warm, 1.0)
        for jj in range(16):
            nc.vector.memset(
                v_sb[jj].rearrange("p (b c) -> p b c", b=2)[:, :, 64:192], 1.0
            )
        if with_qkv_bias:
            ones1 = cons.tile([1, 512], BF16, name="ones1", tag="ones1")
            nc.vector.memset(ones1, 1.0)

        # ---- DMA dispatch: split across the two hw DGE queues -----------
        # sync queue: wq, wk, xt tails, wv, wo (+biases)
        # ACT queue: xt sb0 slices, then xt-odd tails (done before first exp)
        # critical set (wq + wk + xt-sb0, 2MB) dispatched FIRST on both
        # queues: the underlying DMA engines round-robin descriptors from
        # all queues, so any early bulk dispatch (xt tails) delays the
        # weight transfers the head depends on
        dume = cons.tile([1, 64], BF16, name="dume", tag="dume")
        for e in range(8):
            nc.sync.dma_start(out=wq_sb[e], in_=wq_d[e * 128 : (e + 1) * 128, :])
        for e in range(8):
            nc.scalar.dma_start(
                out=xt_sb[e][:, 0:512], in_=xt_d[e * 128 : (e + 1) * 128, 0:512]
            )
        for e in range(4):
            nc.sync.dma_start(out=wk_sb[e], in_=wk_d[e * 128 : (e + 1) * 128, :])
        for e in range(4, 8):
            nc.scalar.dma_start(out=wk_sb[e], in_=wk_d[e * 128 : (e + 1) * 128, :])
        # act-table prewarm off the critical path
        nc.scalar.activation(out=dume, in_=warm[0:1, 0:64], func=EXP, scale=0.01)
        for e in range(0, 8, 2):
            nc.sync.dma_start(
                out=xt_sb[e][:, 512:2048], in_=xt_d[e * 128 : (e + 1) * 128, 512:2048]
            )
        for e in range(1, 8, 2):
            nc.scalar.dma_start(
                out=xt_sb[e][:, 512:2048], in_=xt_d[e * 128 : (e + 1) * 128, 512:2048]
            )
        for e in range(8):
            nc.sync.dma_start(out=wv_sb[e], in_=wv_d[e * 128 : (e + 1) * 128, :])
        for e in range(2):
            nc.sync.dma_start(out=wo_sb[e], in_=wo_d[e * 128 : (e + 1) * 128, :])
        if with_qkv_bias:
            bias_sb = {}
            for nm, d in (("bq", bq_d), ("bk", bk_d), ("bv", bv_d)):
                t = cons.tile([1, DH], BF16, name=f"{nm}s", tag=f"{nm}s")
                nc.sync.dma_start(out=t, in_=d[:, :])
                bias_sb[nm] = t

        # ---- PE warmup (p-state ramp during DMA wait) -------------------
        wps = mip.tile([128, 512], F32, name="wps", tag="mm")
        for i in range(N_WARM):
            nc.tensor.matmul(wps, lhsT=warm[:, 0:128], rhs=warm, start=True, stop=True)

        # ---- emitters ---------------------------------------------------
        qk_state = {}

        def qk_passes(w_sb, bias_nm, dst_sb, pair, sb, es):
            key = (bias_nm, pair, sb)
            if es[0] == 0:
                qk_state[key] = mip.tile(
                    [128, 512], F32, name=f"pqk{bias_nm}{pair}{sb}", tag="mm"
                )
            ps = qk_state[key]
            for e in es:
                nc.tensor.matmul(
                    ps,
                    lhsT=w_sb[e][:, pair * 128 : (pair + 1) * 128],
                    rhs=xt_sb[e][:, sb * 512 : (sb + 1) * 512],
                    start=(e == 0),
                    stop=(e == 7 and not with_qkv_bias),
                )
            if es[-1] == 7:
                if with_qkv_bias:
                    nc.tensor.matmul(
                        ps,
                        lhsT=bias_sb[bias_nm][:, pair * 128 : (pair + 1) * 128],
                        rhs=ones1,
                        start=False,
                        stop=True,
                    )
                nc.vector.tensor_copy(
                    out=dst_sb[pair][:, sb * 512 : (sb + 1) * 512], in_=ps
                )

        v_state = {}

        def v_passes(jj, es):
            if es[0] == 0:
                v_state[jj] = mip.tile([128, DH], F32, name=f"pv{jj}", tag="mm")
            ps = v_state[jj]
            for e in es:
                nc.tensor.matmul(
                    ps,
                    lhsT=xt_sb[e][:, jj * 128 : (jj + 1) * 128],
                    rhs=wv_sb[e],
                    start=(e == 0),
                    stop=(e == 7 and not with_qkv_bias),
                )
            if es[-1] != 7:
                return
            if with_qkv_bias:
                nc.tensor.matmul(
                    ps,
                    lhsT=ones1[:, 0:128],
                    rhs=bias_sb["bv"],
                    start=False,
                    stop=True,
                )
            vv = v_sb[jj].rearrange("p (b c) -> p b c", b=2)
            pp = ps.rearrange("p (h d) -> p h d", h=4)
            nc.vector.tensor_copy(out=vv[:, :, 0:64], in_=pp[:, 0::2, :])
            nc.vector.tensor_copy(out=vv[:, :, 192:256], in_=pp[:, 1::2, :])

        def scores_exp(b, jj):
            pair, ib = b // 4, b % 4
            ps = scp.tile([128, 1024], F32, name=f"sc{b}_{jj}", tag="sc")
            for h in range(2):
                nc.tensor.matmul(
                    ps[:, h * 512 : (h + 1) * 512],
                    lhsT=kt_sb[pair][
                        h * 64 : (h + 1) * 64, jj * 128 : (jj + 1) * 128
                    ],
                    rhs=qt_sb[pair][h * 64 : (h + 1) * 64, ib * 512 : (ib + 1) * 512],
                    start=True,
                    stop=True,
                )
            ex = exq.tile([128, 1024], BF16, name=f"ex{b}_{jj}", tag="ex")
            nc.scalar.activation(out=ex, in_=ps, func=EXP, scale=float(SCALE))
            return ex

        pv_tiles = {}

        def pv_pair(b, jj, ex):
            pair = b // 4
            if jj == 0:
                pv_tiles[b] = pvp.tile([128, 1024], F32, name=f"pvt{b}", tag="pv")
            pv = pv_tiles[b]
            vv = v_sb[jj]
            for h in range(2):
                nc.tensor.matmul(
                    pv[:, h * 512 : (h + 1) * 512],
                    lhsT=vv[:, pair * 256 + h * 128 : pair * 256 + (h + 1) * 128],
                    rhs=ex[:, h * 512 : (h + 1) * 512],
                    start=(jj == 0),
                    stop=(jj == 15),
                )

        epi2_queue = []

        def epilogue(b):
            # h0: ctx rows 0:64, l replicated rows 64:128 (cols 0:512)
            # h1: l replicated rows 0:64, ctx rows 64:128 (cols 512:1024)
            # Emit ONLY the psum-freeing copy here. Tile-pool WAR syncs are
            # coarse per-engine watermarks: any later psum alloc waits for
            # ALL previously-emitted DVE work to retire, so the recip/mul
            # chain (~8us of DVE) must NOT be emitted as a lump at the block
            # boundary -- it is deferred into epi2_queue and dribbled out
            # between subsequent PV pops.
            pair, ib = b // 4, b % 4
            pv = pv_tiles.pop(b)
            pvs = work.tile([128, 1024], F32, name=f"pvs{b}", tag="pvs", bufs=2)
            nc.vector.tensor_copy(out=pvs, in_=pv)
            rlb = work.tile([128, 512], F32, name=f"rl{b}", tag="rl", bufs=2)
            dst = ctxn_sb[pair][ib]

            def recips(q):
                sl = slice(q * 128, (q + 1) * 128)
                nc.vector.reciprocal(out=rlb[0:64, sl], in_=pvs[64:128, sl])
                nc.vector.reciprocal(
                    out=rlb[64:128, sl],
                    in_=pvs[0:64, 512 + q * 128 : 512 + (q + 1) * 128],
                )

            def muls(h):
                if h == 0:
                    nc.vector.tensor_mul(
                        out=dst[0:64, :], in0=rlb[0:64, :], in1=pvs[0:64, 0:512]
                    )
                else:
                    nc.vector.tensor_mul(
                        out=dst[64:128, :],
                        in0=rlb[64:128, :],
                        in1=pvs[64:128, 512:1024],
                    )

            atoms = [lambda q=q: recips(q) for q in range(4)]
            atoms += [lambda: muls(0), lambda: muls(1)]
            epi2_queue.append((b, atoms))

        fill_hold = [-1]

        def drain_epi2(n, t):
            while n > 0 and epi2_queue:
                b, atoms = epi2_queue[0]
                atoms.pop(0)()
                n -= 1
                if not atoms:
                    epi2_queue.pop(0)
                    # suppression must outlive EMISSION of the last atoms:
                    # the muls just queued still take ~2us to EXECUTE on the
                    # DVE, and a fresh psum alloc emitted before they retire
                    # watermark-blocks the PE with ready scores stuck behind
                    fill_hold[0] = t + 3
                    if b >= 4:
                        add_outproj(b - 4)

        def epilogue_tail(b):
            # final block: no next block needs the psum banks; read pv
            # directly and pipeline per-quarter so outproj(3, ss) starts as
            # soon as its ctxn columns are ready. h1's reciprocal runs as
            # ln/exp on the ACT engine (idle after the last exp) so the DVE
            # chain halves; the +64 partition shift rides on a DVE copy
            # (the one shift direction plain copies support on hw).
            pair, ib = b // 4, b % 4
            pv = pv_tiles.pop(b)
            rlb = work.tile([128, 512], F32, name=f"rl{b}", tag="rl", bufs=2)
            dst = ctxn_sb[pair][ib]
            for q in range(4):
                sl = slice(q * 128, (q + 1) * 128)
                sh = slice(512 + q * 128, 512 + (q + 1) * 128)
                nc.vector.reciprocal(out=rlb[0:64, sl], in_=pv[64:128, sl])
                nc.vector.reciprocal(out=rlb[64:128, sl], in_=pv[0:64, sh])
                nc.vector.tensor_mul(
                    out=dst[0:64, sl], in0=rlb[0:64, sl], in1=pv[0:64, sl]
                )
                nc.vector.tensor_mul(
                    out=dst[64:128, sl], in0=rlb[64:128, sl], in1=pv[64:128, sh]
                )
                for eb in range(2):
                    op_group(ib, q, eb, tail=True, dmaq=(q + eb) % 2)

        def op_group(ib, ss, eb, tail=False, dmaq=1):
            po = mip.tile([128, 512], F32, name=f"po{ib}{ss}{eb}", tag="mm")
            for cc in range(2):
                nc.tensor.matmul(
                    po,
                    lhsT=ctxn_sb[cc][ib][:, ss * 128 : (ss + 1) * 128],
                    rhs=wo_sb[cc][:, eb * 512 : (eb + 1) * 512],
                    start=(cc == 0),
                    stop=(cc == 1),
                )
            ot = work.tile([128, 512], F32, name=f"ot{ib}{ss}{eb}", tag="ot", bufs=4)
            if tail:
                # ACT is idle after the last exp: use it for the final
                # copies and DMA dispatch so they run parallel to the DVE
                # epilogue instead of behind it
                nc.scalar.activation(
                    out=ot, in_=po, func=mybir.ActivationFunctionType.Copy
                )
            else:
                nc.vector.tensor_copy(out=ot, in_=po)
            row = ib * 512 + ss * 128
            dq = (nc.scalar if dmaq else nc.sync) if tail else nc.sync
            dq.dma_start(
                out=out_d[row : row + 128, eb * 512 : (eb + 1) * 512], in_=ot
            )

        # ---- fill machinery --------------------------------------------
        # fill groups: list of atoms (cost, fn); groups with psum usage are
        # marked so at most 2 are open at once (mip bufs=2).
        class Group:
            __slots__ = ("deadline", "atoms", "idx", "psum", "soft")

            def __init__(self, deadline, atoms, psum, soft=False):
                self.deadline = deadline
                self.atoms = atoms
                self.idx = 0
                self.psum = psum
                self.soft = soft  # deadline orders EDF only; never force-drained

            def done(self):
                return self.idx >= len(self.atoms)

        fills = []

        def add_qk(w_sb, bias_nm, dst_sb, pair, sb, deadline):
            fills.append(
                Group(
                    deadline,
                    [
                        (
                            2 * COST_QK,
                            lambda es=es: qk_passes(w_sb, bias_nm, dst_sb, pair, sb, es),
                        )
                        for es in ([0, 1], [2, 3], [4, 5], [6, 7])
                    ],
                    True,
                )
            )

        v_groups = {}
        for jj in range(16):
            g = Group(
                20 + jj,
                [
                    (4 * COST_V, lambda jj=jj: v_passes(jj, [0, 1, 2, 3])),
                    (4 * COST_V, lambda jj=jj: v_passes(jj, [4, 5, 6, 7])),
                ],
                True,
                soft=True,
            )
            v_groups[jj] = g
            fills.append(g)
        # pair-0 q/k beyond sb0 (sb0 is the head)
        for sb in range(1, 4):
            add_qk(wk_sb, "bk", kt_sb, 0, sb, 4 * sb)
        for sb in range(1, 4):
            add_qk(wq_sb, "bq", qt_sb, 0, sb, 16 * sb)
        # pair-1 q/k
        for sb in range(4):
            add_qk(wk_sb, "bk", kt_sb, 1, sb, 64 + 4 * sb)
        for sb in range(4):
            add_qk(wq_sb, "bq", qt_sb, 1, sb, 64 + 16 * sb)

        def add_outproj(ib):
            for ss in range(4):
                fills.append(
                    Group(
                        300 + ib * 10 + ss,
                        [
                            (COST_OP, lambda ss=ss, eb=0: op_group(ib, ss, eb)),
                            (COST_OP, lambda ss=ss, eb=1: op_group(ib, ss, eb)),
                        ],
                        True,
                    )
                )

        # ---- the scheduler ---------------------------------------------
        state = {"prefix": 0.0, "open": []}

        def emit_atom(g):
            cost, fn = g.atoms[g.idx]
            g.idx += 1
            fn()
            state["prefix"] += cost
            if g.psum:
                if g not in state["open"]:
                    state["open"].append(g)
                if g.done():
                    state["open"].remove(g)
            if g.done():
                fills.remove(g)

        def pickable(g):
            # at most 2 open psum groups; prefer finishing open ones
            if not g.psum:
                return True
            opens = [o for o in state["open"] if not o.done()]
            return g in opens or len(opens) < 2

        def drain_mandatory(t):
            # smooth: pull deadline work early but only as many atoms per
            # step as needed to finish by the deadline (whole-group dumps
            # create 1.7us emission lumps that delay the next scores).
            # Look ahead 8 steps, but while an epilogue-drain hold is active
            # defer any group that still has >2 steps of slack: a mandatory
            # psum alloc emitted mid-hold watermark-blocks the PE queue.
            held = bool(epi2_queue) or t < fill_hold[0]
            for g in sorted(fills, key=lambda g: g.deadline):
                if g.soft or g.deadline > t + 8:
                    continue
                if held and g.deadline > t + 2 and g.idx == 0:
                    continue
                remaining = len(g.atoms) - g.idx
                steps_left = max(1, g.deadline - t - 1)
                k = -(-remaining // steps_left)  # ceil
                for _ in range(k):
                    if not g.done():
                        emit_atom(g)

        def pop_fill(cont_only=False):
            for g in sorted(fills, key=lambda g: g.deadline):
                if cont_only and g.psum and g.idx == 0:
                    # during epilogue drains / pv pauses, only continue
                    # already-open groups: a fresh psum alloc would
                    # watermark-wait on the DVE backlog
                    continue
                if pickable(g):
                    emit_atom(g)
                    return True
            return False

        pv_pending = []  # (b, jj, ex, step_scored)
        pv_block_open = [None]  # current block being PV'd
        pv_resume = [0]

        def pv_target(t):
            # keep a reserve of poppable PV work through the mid/late phase
            # so block-boundary (epilogue-drain) steps and fill-dry stretches
            # always have PE work; taper to zero near the end so no PV tail
            # remains after the last scores
            if t < 24:
                return 0
            if t < 96:
                return 12
            return max(0, 12 - (t - 96) * 12 // 30)

        def pop_pv(t, force=False):
            if not pv_pending:
                return False
            b, jj, ex, ts = pv_pending[0]
            if force and len(pv_pending) < NEX - 2 and t < pv_resume[0]:
                # respect the pv-bank WAR pause unless the ex pool is at its
                # hard ceiling
                return False
            if not force:
                if t < pv_resume[0]:
                    return False
                # during epilogue drains fills are suppressed; relax the
                # backlog floor so PV pops can keep the steps thick
                floor = pv_target(t) - (
                    6 if (epi2_queue or t < fill_hold[0]) else 0
                )
                if len(pv_pending) <= floor:
                    return False
                minage = MINAGE_B0 if b == 0 else MINAGE
                if t - ts < minage:
                    return False
            # v must be emitted before PV(0, jj)
            if b == 0:
                g = v_groups[jj]
                while not g.done():
                    emit_atom(g)
            pv_pending.pop(0)
            pv_pair(b, jj, ex)
            state["prefix"] += COST_PV
            if jj == 15:
                if b == 7:
                    while epi2_queue:
                        drain_epi2(99, NSTEPS)
                    # flush remaining fills (outproj leftovers) BEFORE the
                    # final epilogue: they run on the PE in parallel with its
                    # DVE reciprocal chain instead of queueing behind it
                    while fills and pop_fill():
                        pass
                    epilogue_tail(b)
                else:
                    epilogue(b)
                pv_resume[0] = t + PV_PAUSE
            return True

        # ---- head: q/k (pair0, sb0) ------------------------------------
        qk_passes(wq_sb, "bq", qt_sb, 0, 0, list(range(8)))
        qk_passes(wk_sb, "bk", kt_sb, 0, 0, list(range(8)))
        state["prefix"] = 0.0

        # ---- main loop --------------------------------------------------
        for t in range(NSTEPS):
            b, jj = t // JPB, t % JPB
            step_base = state["prefix"]
            ex = scores_exp(b, jj)
            pv_pending.append((b, jj, ex, t))
            state["prefix"] += COST_SCORE
            # one deferred-epilogue DVE atom per step: matches DVE drain rate
            # so the DVE backlog at any later psum alloc stays ~1 item deep
            drain_epi2(1, t)
            drain_mandatory(t)
            budget = (t + 1) * X_PACE
            while True:
                if len(pv_pending) > MAXLAG:
                    if pop_pv(t, force=True):
                        continue
                over = state["prefix"] >= budget
                # min-work floor: even over budget, keep >= MIN_STEP of PE
                # work per step so the PE queue between consecutive scores
                # never thins out (a thin stretch lets the PE race into the
                # scores-psum WAR and stall with ready work queued behind)
                if over and state["prefix"] - step_base >= MIN_STEP:
                    break
                if pop_pv(t):
                    continue
                if pop_fill(
                    cont_only=bool(epi2_queue)
                    or t < fill_hold[0]
                    or t < pv_resume[0]
                ):
                    continue
                break

        # ---- tail -------------------------------------------------------
        while pv_pending:
            jj = pv_pending[0][1]
            pop_pv(NSTEPS, force=True)
            if jj == 15:
                # give the pv-bank WAR (epilogue DVE chain) room: interleave
                # outproj work between blocks
                for _ in range(6):
                    if fills:
                        pop_fill()
        while fills:
            if not pop_fill():
                # only blocked psum groups remain; force the first
                emit_atom(sorted(fills, key=lambda g: g.deadline)[0])

    nc.compile()
    return nc


def _get_graph(with_qkv_bias: bool):
    key = ("nc", with_qkv_bias)
    if key not in _CACHE:
        _CACHE[key] = _build_graph(with_qkv_bias)
    return _CACHE[key]


def make_in_maps(x, Wq, bq, Wk, bk, Wv, bv, Wo, with_qkv_bias):
    bf16 = ml_dtypes.bfloat16
    in_maps = []
    for c in range(NCORES):
        b, g = c // GROUPS, c % GROUPS
        hs = slice(g * DH, (g + 1) * DH)
        m = {
            "xt": np.ascontiguousarray(x[b].T.astype(bf16)),
            "wq": np.ascontiguousarray(Wq[hs, :].T.astype(bf16)),
            "wk": np.ascontiguousarray(Wk[hs, :].T.astype(bf16)),
            "wv": np.ascontiguousarray(Wv[hs, :].T.astype(bf16)),
            "wo": np.ascontiguousarray(Wo[:, hs].T.astype(bf16)),
        }
        if with_qkv_bias:
            m["bq"] = np.ascontiguousarray(bq[None, hs].astype(bf16))
            m["bk"] = np.ascontiguousarray(bk[None, hs].astype(bf16))
            m["bv"] = np.ascontiguousarray(bv[None, hs].astype(bf16))
        in_maps.append(m)
    return in_maps


def kernel(x, Wq, bq, Wk, bk, Wv, bv, Wo, bo):
    global last_exec_time_ns, last_results
    from concourse.bass_utils import run_bass_kernel_spmd

    x = np.asarray(x, np.float32)
    Wq = np.asarray(Wq, np.float32)
    Wk = np.asarray(Wk, np.float32)
    Wv = np.asarray(Wv, np.float32)
    Wo = np.asarray(Wo, np.float32)
    bq = np.asarray(bq, np.float32)
    bk = np.asarray(bk, np.float32)
    bv = np.asarray(bv, np.float32)
    bo = np.asarray(bo, np.float32)

    with_qkv_bias = bool(np.any(bq) or np.any(bk) or np.any(bv))
    nc = _get_graph(with_qkv_bias)
    in_maps = make_in_maps(x, Wq, bq, Wk, bk, Wv, bv, Wo, with_qkv_bias)

    trace = os.environ.get("BASS_KERNEL_TRACE", "0") == "1"
    tdir = os.environ.get("BASS_KERNEL_TRACE_DIR") or None
    res = run_bass_kernel_spmd(
        nc, in_maps, list(range(NCORES)), trace=trace, tmpdir=tdir
    )
    last_exec_time_ns = res.exec_time_ns
    last_results = res

    out = np.zeros((B, S, HID), np.float32)
    for c in range(NCORES):
        out[c // GROUPS] += np.asarray(res.results[c]["out"], np.float32)
    out += bo
    return out


# revision 71
# speedup vs baseline: 1.0327x; 1.0327x over previous
"""Multi-head attention (B=2, S=2048, H=1024, 16 heads) on 8 TRN2 NeuronCores.

Sharding: core c -> batch b = c//4, head-group g = c%4 (heads 4g..4g+3).
Each core computes q/k/v projections for its 4 heads (tensor parallel),
full attention for those heads, and a partial output projection
(contribution of its 256 hidden dims). Host sums the 4 partials per batch
and adds the output bias.

v2 schedule: the ACT engine (128 exps of [128,1024], ~1.1us each) is the
hard floor (~142us); PE work (~640 x 512-col passes ~ 137us) is list-
scheduled into a single budget-paced stream so the PE never idles (idle
drops the PE p-state to 1.2GHz for ~3us) and every scores tile is ready
ahead of its exp.

Key kernel tricks vs the naive layout:
  - scores: two K=64 matmuls row-packed at partition base 0/64 run
    concurrently (both heads of a pair in one 512-col pass).
  - PV stationary per head is [v_h | ones*64] ([1s*64 | v_h] for the odd
    head): M=128 costs the same N cycles as M=65, and the softmax row-sum
    l comes out of PSUM already replicated across 64 partitions.
  - epilogue per (pair, ib): DVE reciprocal_approx_fast on the replicated
    l rows (PSUM in, partition-shifted out), then two DVE multiplies
    pv*rl -> ctxn bf16. No ACT work, no broadcast matmuls, no shift DMAs.
  - PE warmup matmuls + a dummy exp during the initial DMA wait (p-state
    ramp + act-table load off the critical path).
  - DMA dispatch split across the two hardware DGE queues (sync + ACT).
"""

import os
from contextlib import ExitStack

import numpy as np
import ml_dtypes

B = 2
S = 2048
HID = 1024
NHEAD = 16
HDIM = 64
NCORES = 8
GROUPS = 4  # head-groups per batch (cores per batch)
DH = 256  # hidden dims per core (4 heads x 64)
SCALE = 1.0 / np.sqrt(np.float32(HDIM))  # 0.125

NSTEPS = 128  # 8 blocks x 16 j-chunks
JPB = 16

# scheduler tunables (us of PE work per atom; X = ACT pace per step)
COST_SCORE = 0.225
COST_PV = 0.43
COST_QK = 0.215
COST_V = 0.115
COST_OP = 0.43
X_PACE = 1.10
MIN_STEP = 0.75
MINAGE_B0 = 8
MINAGE = 2
MAXLAG = 22
NEX = 26
PV_PAUSE = 1  # steps to pause PV pops after a block's last PV (pv bank WAR)
N_WARM = 12

_CACHE = {}
last_exec_time_ns = None
last_results = None


def _build_graph(with_qkv_bias: bool):
    import concourse.bass as bass
    import concourse.mybir as mybir
    import concourse.tile as tile
    from concourse import bacc

    F32 = mybir.dt.float32
    BF16 = mybir.dt.bfloat16
    EXP = mybir.ActivationFunctionType.Exp

    nc = bacc.Bacc()
    xt_d = nc.declare_dram_parameter("xt", [HID, S], BF16, isOutput=False)
    wq_d = nc.declare_dram_parameter("wq", [HID, DH], BF16, isOutput=False)
    wk_d = nc.declare_dram_parameter("wk", [HID, DH], BF16, isOutput=False)
    wv_d = nc.declare_dram_parameter("wv", [HID, DH], BF16, isOutput=False)
    wo_d = nc.declare_dram_parameter("wo", [DH, HID], BF16, isOutput=False)
    if with_qkv_bias:
        bq_d = nc.declare_dram_parameter("bq", [1, DH], BF16, isOutput=False)
        bk_d = nc.declare_dram_parameter("bk", [1, DH], BF16, isOutput=False)
        bv_d = nc.declare_dram_parameter("bv", [1, DH], BF16, isOutput=False)
    out_d = nc.declare_dram_parameter("out", [S, HID], F32, isOutput=True)

    with ExitStack() as ctx:
        tc = ctx.enter_context(tile.TileContext(nc))
        cons = ctx.enter_context(tc.tile_pool(name="cons", bufs=1))
        work = ctx.enter_context(tc.tile_pool(name="work", bufs=3))
        exq = ctx.enter_context(tc.tile_pool(name="exq", bufs=NEX))
        scp = ctx.enter_context(tc.tile_pool(name="scp", bufs=2, space="PSUM"))
        pvp = ctx.enter_context(tc.tile_pool(name="pvp", bufs=1, space="PSUM"))
        mip = ctx.enter_context(tc.tile_pool(name="mip", bufs=2, space="PSUM"))

        # ---- persistent tiles -------------------------------------------
        xt_sb = [
            cons.tile([128, S], BF16, name=f"xt{e}", tag=f"xt{e}") for e in range(8)
        ]
        wq_sb = [
            cons.tile([128, DH], BF16, name=f"wq{e}", tag=f"wq{e}") for e in range(8)
        ]
        wk_sb = [
            cons.tile([128, DH], BF16, name=f"wk{e}", tag=f"wk{e}") for e in range(8)
        ]
        wv_sb = [
            cons.tile([128, DH], BF16, name=f"wv{e}", tag=f"wv{e}") for e in range(8)
        ]
        wo_sb = [
            cons.tile([128, HID], BF16, name=f"wo{e}", tag=f"wo{e}") for e in range(2)
        ]
        qt_sb = [
            cons.tile([128, S], BF16, name=f"qt{c}", tag=f"qt{c}") for c in range(2)
        ]
        kt_sb = [
            cons.tile([128, S], BF16, name=f"kt{c}", tag=f"kt{c}") for c in range(2)
        ]
        # v per j-chunk: two 256-col pair blocks [v_even | ones128 | v_odd]
        v_sb = [
            cons.tile([128, 512], BF16, name=f"v{j}", tag=f"v{j}") for j in range(16)
        ]
        ctxn_sb = [
            [
                cons.tile([128, 512], BF16, name=f"cx{c}_{i}", tag=f"cx{c}_{i}")
                for i in range(4)
            ]
            for c in range(2)
        ]

        # ---- memsets (engines idle during DMA wait) ---------------------
        warm = cons.tile([128, 512], BF16, name="warm", tag="warm")
        nc.vector.memset(warm, 1.0)
        for jj in range(16):
            nc.vector.memset(
                v_sb[jj].rearrange("p (b c) -> p b c", b=2)[:, :, 64:192], 1.0
            )
        if with_qkv_bias:
            ones1 = cons.tile([1, 512], BF16, name="ones1", tag="ones1")
            nc.vector.memset(ones1, 1.0)

        # ---- DMA dispatch: split across the two hw DGE queues -----------
        # sync queue: wq, wk, xt tails, wv, wo (+biases)
        # ACT queue: xt sb0 slices, then xt-odd tails (done before first exp)
        # critical set (wq + wk + xt-sb0, 2MB) dispatched FIRST on both
        # queues: the underlying DMA engines round-robin descriptors from
        # all queues, so any early bulk dispatch (xt tails) delays the
        # weight transfers the head depends on
        dume = cons.tile([1, 64], BF16, name="dume", tag="dume")
        for e in range(8):
            nc.sync.dma_start(out=wq_sb[e], in_=wq_d[e * 128 : (e + 1) * 128, :])
        for e in range(8):
            nc.scalar.dma_start(
                out=xt_sb[e][:, 0:512], in_=xt_d[e * 128 : (e + 1) * 128, 0:512]
            )
        for e in range(4):
            nc.sync.dma_start(out=wk_sb[e], in_=wk_d[e * 128 : (e + 1) * 128, :])
        for e in range(4, 8):
            nc.scalar.dma_start(out=wk_sb[e], in_=wk_d[e * 128 : (e + 1) * 128, :])
        # act-table prewarm off the critical path
        nc.scalar.activation(out=dume, in_=warm[0:1, 0:64], func=EXP, scale=0.01)
        for e in range(0, 8, 2):
            nc.sync.dma_start(
                out=xt_sb[e][:, 512:2048], in_=xt_d[e * 128 : (e + 1) * 128, 512:2048]
            )
        for e in range(1, 8, 2):
            nc.scalar.dma_start(
                out=xt_sb[e][:, 512:2048], in_=xt_d[e * 128 : (e + 1) * 128, 512:2048]
            )
        for e in range(8):
            nc.sync.dma_start(out=wv_sb[e], in_=wv_d[e * 128 : (e + 1) * 128, :])
        for e in range(2):
            nc.sync.dma_start(out=wo_sb[e], in_=wo_d[e * 128 : (e + 1) * 128, :])
        if with_qkv_bias:
            bias_sb = {}
            for nm, d in (("bq", bq_d), ("bk", bk_d), ("bv", bv_d)):
                t = cons.tile([1, DH], BF16, name=f"{nm}s", tag=f"{nm}s")
                nc.sync.dma_start(out=t, in_=d[:, :])
                bias_sb[nm] = t

        # ---- PE warmup (p-state ramp during DMA wait) -------------------
        wps = mip.tile([128, 512], F32, name="wps", tag="mm")
        for i in range(N_WARM):
            nc.tensor.matmul(wps, lhsT=warm[:, 0:128], rhs=warm, start=True, stop=True)

        # ---- emitters ---------------------------------------------------
        qk_state = {}

        def qk_passes(w_sb, bias_nm, dst_sb, pair, sb, es):
            key = (bias_nm, pair, sb)
            if es[0] == 0:
                qk_state[key] = mip.tile(
                    [128, 512], F32, name=f"pqk{bias_nm}{pair}{sb}", tag="mm"
                )
            ps = qk_state[key]
            for e in es:
                nc.tensor.matmul(
                    ps,
                    lhsT=w_sb[e][:, pair * 128 : (pair + 1) * 128],
                    rhs=xt_sb[e][:, sb * 512 : (sb + 1) * 512],
                    start=(e == 0),
                    stop=(e == 7 and not with_qkv_bias),
                )
            if es[-1] == 7:
                if with_qkv_bias:
                    nc.tensor.matmul(
                        ps,
                        lhsT=bias_sb[bias_nm][:, pair * 128 : (pair + 1) * 128],
                        rhs=ones1,
                        start=False,
                        stop=True,
                    )
                nc.vector.tensor_copy(
                    out=dst_sb[pair][:, sb * 512 : (sb + 1) * 512], in_=ps
                )

        v_state = {}

        def v_passes(jj, es):
            if es[0] == 0:
                v_state[jj] = mip.tile([128, DH], F32, name=f"pv{jj}", tag="mm")
            ps = v_state[jj]
            for e in es:
                nc.tensor.matmul(
                    ps,
                    lhsT=xt_sb[e][:, jj * 128 : (jj + 1) * 128],
                    rhs=wv_sb[e],
                    start=(e == 0),
                    stop=(e == 7 and not with_qkv_bias),
                )
            if es[-1] != 7:
                return
            if with_qkv_bias:
                nc.tensor.matmul(
                    ps,
                    lhsT=ones1[:, 0:128],
                    rhs=bias_sb["bv"],
                    start=False,
                    stop=True,
                )
            vv = v_sb[jj].rearrange("p (b c) -> p b c", b=2)
            pp = ps.rearrange("p (h d) -> p h d", h=4)
            nc.vector.tensor_copy(out=vv[:, :, 0:64], in_=pp[:, 0::2, :])
            nc.vector.tensor_copy(out=vv[:, :, 192:256], in_=pp[:, 1::2, :])

        def scores_exp(b, jj):
            pair, ib = b // 4, b % 4
            ps = scp.tile([128, 1024], F32, name=f"sc{b}_{jj}", tag="sc")
            for h in range(2):
                nc.tensor.matmul(
                    ps[:, h * 512 : (h + 1) * 512],
                    lhsT=kt_sb[pair][
                        h * 64 : (h + 1) * 64, jj * 128 : (jj + 1) * 128
                    ],
                    rhs=qt_sb[pair][h * 64 : (h + 1) * 64, ib * 512 : (ib + 1) * 512],
                    start=True,
                    stop=True,
                )
            ex = exq.tile([128, 1024], BF16, name=f"ex{b}_{jj}", tag="ex")
            nc.scalar.activation(out=ex, in_=ps, func=EXP, scale=float(SCALE))
            return ex

        pv_tiles = {}

        def pv_pair(b, jj, ex):
            pair = b // 4
            if jj == 0:
                pv_tiles[b] = pvp.tile([128, 1024], F32, name=f"pvt{b}", tag="pv")
            pv = pv_tiles[b]
            vv = v_sb[jj]
            for h in range(2):
                nc.tensor.matmul(
                    pv[:, h * 512 : (h + 1) * 512],
                    lhsT=vv[:, pair * 256 + h * 128 : pair * 256 + (h + 1) * 128],
                    rhs=ex[:, h * 512 : (h + 1) * 512],
                    start=(jj == 0),
                    stop=(jj == 15),
                )

        epi2_queue = []

        def epilogue(b):
            # h0: ctx rows 0:64, l replicated rows 64:128 (cols 0:512)
            # h1: l replicated rows 0:64, ctx rows 64:128 (cols 512:1024)
            # Emit ONLY the psum-freeing copy here. Tile-pool WAR syncs are
            # coarse per-engine watermarks: any later psum alloc waits for
            # ALL previously-emitted DVE work to retire, so the recip/mul
            # chain (~8us of DVE) must NOT be emitted as a lump at the block
            # boundary -- it is deferred into epi2_queue and dribbled out
            # between subsequent PV pops.
            pair, ib = b // 4, b % 4
            pv = pv_tiles.pop(b)
            pvs = work.tile([128, 1024], F32, name=f"pvs{b}", tag="pvs", bufs=2)
            nc.vector.tensor_copy(out=pvs, in_=pv)
            rlb = work.tile([128, 512], F32, name=f"rl{b}", tag="rl", bufs=2)
            dst = ctxn_sb[pair][ib]

            def recips(q):
                sl = slice(q * 128, (q + 1) * 128)
                nc.vector.reciprocal(out=rlb[0:64, sl], in_=pvs[64:128, sl])
                nc.vector.reciprocal(
                    out=rlb[64:128, sl],
                    in_=pvs[0:64, 512 + q * 128 : 512 + (q + 1) * 128],
                )

            def muls(h):
                if h == 0:
                    nc.vector.tensor_mul(
                        out=dst[0:64, :], in0=rlb[0:64, :], in1=pvs[0:64, 0:512]
                    )
                else:
                    nc.vector.tensor_mul(
                        out=dst[64:128, :],
                        in0=rlb[64:128, :],
                        in1=pvs[64:128, 512:1024],
                    )

            atoms = [lambda q=q: recips(q) for q in range(4)]
            atoms += [lambda: muls(0), lambda: muls(1)]
            epi2_queue.append((b, atoms))

        fill_hold = [-1]

        def drain_epi2(n, t):
            while n > 0 and epi2_queue:
                b, atoms = epi2_queue[0]
                atoms.pop(0)()
                n -= 1
                if not atoms:
                    epi2_queue.pop(0)
                    # suppression must outlive EMISSION of the last atoms:
                    # the muls just queued still take ~2us to EXECUTE on the
                    # DVE, and a fresh psum alloc emitted before they retire
                    # watermark-blocks the PE with ready scores stuck behind
                    fill_hold[0] = t + 3
                    if b >= 4:
                        add_outproj(b - 4)

        def epilogue_tail(b):
            # final block: no next block needs the psum banks; read pv
            # directly and pipeline per-quarter so outproj(3, ss) starts as
            # soon as its ctxn columns are ready. h1's reciprocal runs as
            # ln/exp on the ACT engine (idle after the last exp) so the DVE
            # chain halves; the +64 partition shift rides on a DVE copy
            # (the one shift direction plain copies support on hw).
            pair, ib = b // 4, b % 4
            pv = pv_tiles.pop(b)
            rlb = work.tile([128, 512], F32, name=f"rl{b}", tag="rl", bufs=2)
            dst = ctxn_sb[pair][ib]
            for q in range(4):
                sl = slice(q * 128, (q + 1) * 128)
                sh = slice(512 + q * 128, 512 + (q + 1) * 128)
                nc.vector.reciprocal(out=rlb[0:64, sl], in_=pv[64:128, sl])
                nc.vector.reciprocal(out=rlb[64:128, sl], in_=pv[0:64, sh])
                nc.vector.tensor_mul(
                    out=dst[0:64, sl], in0=rlb[0:64, sl], in1=pv[0:64, sl]
                )
                nc.vector.tensor_mul(
                    out=dst[64:128, sl], in0=rlb[64:128, sl], in1=pv[64:128, sh]
                )
                for eb in range(2):
                    op_group(ib, q, eb, tail=True, dmaq=(q + eb) % 2)

        def op_group(ib, ss, eb, tail=False, dmaq=1):
            po = mip.tile([128, 512], F32, name=f"po{ib}{ss}{eb}", tag="mm")
            for cc in range(2):
                nc.tensor.matmul(
                    po,
                    lhsT=ctxn_sb[cc][ib][:, ss * 128 : (ss + 1) * 128],
                    rhs=wo_sb[cc][:, eb * 512 : (eb + 1) * 512],
                    start=(cc == 0),
                    stop=(cc == 1),
                )
            ot = work.tile([128, 512], F32, name=f"ot{ib}{ss}{eb}", tag="ot", bufs=4)
            if tail:
                # ACT is idle after the last exp: use it for the final
                # copies and DMA dispatch so they run parallel to the DVE
                # epilogue instead of behind it
                nc.scalar.activation(
                    out=ot, in_=po, func=mybir.ActivationFunctionType.Copy
                )
            else:
                nc.vector.tensor_copy(out=ot, in_=po)
            row = ib * 512 + ss * 128
            dq = (nc.scalar if dmaq else nc.sync) if tail else nc.sync
            dq.dma_start(
                out=out_d[row : row + 128, eb * 512 : (eb + 1) * 512], in_=ot
            )

        # ---- fill machinery --------------------------------------------
        # fill groups: list of atoms (cost, fn); groups with psum usage are
        # marked so at most 2 are open at once (mip bufs=2).
        class Group:
            __slots__ = ("deadline", "atoms", "idx", "psum", "soft")

            def __init__(self, deadline, atoms, psum, soft=False):
                self.deadline = deadline
                self.atoms = atoms
                self.idx = 0
                self.psum = psum
                self.soft = soft  # deadline orders EDF only; never force-drained

            def done(self):
                return self.idx >= len(self.atoms)

        fills = []

        def add_qk(w_sb, bias_nm, dst_sb, pair, sb, deadline):
            fills.append(
                Group(
                    deadline,
                    [
                        (
                            2 * COST_QK,
                            lambda es=es: qk_passes(w_sb, bias_nm, dst_sb, pair, sb, es),
                        )
                        for es in ([0, 1], [2, 3], [4, 5], [6, 7])
                    ],
                    True,
                )
            )

        v_groups = {}
        for jj in range(16):
            g = Group(
                20 + jj,
                [
                    (4 * COST_V, lambda jj=jj: v_passes(jj, [0, 1, 2, 3])),
                    (4 * COST_V, lambda jj=jj: v_passes(jj, [4, 5, 6, 7])),
                ],
                True,
                soft=True,
            )
            v_groups[jj] = g
            fills.append(g)
        # pair-0 q/k beyond sb0 (sb0 is the head)
        for sb in range(1, 4):
            add_qk(wk_sb, "bk", kt_sb, 0, sb, 4 * sb)
        for sb in range(1, 4):
            add_qk(wq_sb, "bq", qt_sb, 0, sb, 16 * sb)
        # pair-1 q/k
        for sb in range(4):
            add_qk(wk_sb, "bk", kt_sb, 1, sb, 64 + 4 * sb)
        for sb in range(4):
            add_qk(wq_sb, "bq", qt_sb, 1, sb, 64 + 16 * sb)

        def add_outproj(ib):
            for ss in range(4):
                fills.append(
                    Group(
                        300 + ib * 10 + ss,
                        [
                            (COST_OP, lambda ss=ss, eb=0: op_group(ib, ss, eb)),
                            (COST_OP, lambda ss=ss, eb=1: op_group(ib, ss, eb)),
                        ],
                        True,
                    )
                )

        # ---- the scheduler ---------------------------------------------
        state = {"prefix": 0.0, "open": []}

        def emit_atom(g):
            cost, fn = g.atoms[g.idx]
            g.idx += 1
            fn()
            state["prefix"] += cost
            if g.psum:
                if g not in state["open"]:
                    state["open"].append(g)
                if g.done():
                    state["open"].remove(g)
            if g.done():
                fills.remove(g)

        def pickable(g):
            # at most 2 open psum groups; prefer finishing open ones
            if not g.psum:
                return True
            opens = [o for o in state["open"] if not o.done()]
            return g in opens or len(opens) < 2

        def drain_mandatory(t):
            # smooth: pull deadline work early but only as many atoms per
            # step as needed to finish by the deadline (whole-group dumps
            # create 1.7us emission lumps that delay the next scores)
            for g in sorted(fills, key=lambda g: g.deadline):
                if g.soft or g.deadline > t + 4:
                    continue
                remaining = len(g.atoms) - g.idx
                steps_left = max(1, g.deadline - t - 1)
                k = -(-remaining // steps_left)  # ceil
                for _ in range(k):
                    if not g.done():
                        emit_atom(g)

        def pop_fill(cont_only=False):
            for g in sorted(fills, key=lambda g: g.deadline):
                if cont_only and g.psum and g.idx == 0:
                    # during epilogue drains / pv pauses, only continue
                    # already-open groups: a fresh psum alloc would
                    # watermark-wait on the DVE backlog
                    continue
                if pickable(g):
                    emit_atom(g)
                    return True
            return False

        pv_pending = []  # (b, jj, ex, step_scored)
        pv_block_open = [None]  # current block being PV'd
        pv_resume = [0]

        def pv_target(t):
            # keep a reserve of poppable PV work through the mid/late phase
            # so block-boundary (epilogue-drain) steps and fill-dry stretches
            # always have PE work; taper to zero near the end so no PV tail
            # remains after the last scores
            if t < 24:
                return 0
            if t < 96:
                return 12
            return max(0, 12 - (t - 96) * 12 // 30)

        def pop_pv(t, force=False):
            if not pv_pending:
                return False
            b, jj, ex, ts = pv_pending[0]
            if force and len(pv_pending) < NEX - 2 and t < pv_resume[0]:
                # respect the pv-bank WAR pause unless the ex pool is at its
                # hard ceiling
                return False
            if not force:
                if t < pv_resume[0]:
                    return False
                # during epilogue drains fills are suppressed; relax the
                # backlog floor so PV pops can keep the steps thick
                floor = pv_target(t) - (
                    6 if (epi2_queue or t < fill_hold[0]) else 0
                )
                if len(pv_pending) <= floor:
                    return False
                minage = MINAGE_B0 if b == 0 else MINAGE
                if t - ts < minage:
                    return False
            # v must be emitted before PV(0, jj)
            if b == 0:
                g = v_groups[jj]
                while not g.done():
                    emit_atom(g)
            pv_pending.pop(0)
            pv_pair(b, jj, ex)
            state["prefix"] += COST_PV
            if jj == 15:
                if b == 7:
                    while epi2_queue:
                        drain_epi2(99, NSTEPS)
                    # flush remaining fills (outproj leftovers) BEFORE the
                    # final epilogue: they run on the PE in parallel with its
                    # DVE reciprocal chain instead of queueing behind it
                    while fills and pop_fill():
                        pass
                    epilogue_tail(b)
                else:
                    epilogue(b)
                pv_resume[0] = t + PV_PAUSE
            return True

        # ---- head: q/k (pair0, sb0) ------------------------------------
        qk_passes(wq_sb, "bq", qt_sb, 0, 0, list(range(8)))
        qk_passes(wk_sb, "bk", kt_sb, 0, 0, list(range(8)))
        state["prefix"] = 0.0

        # ---- main loop --------------------------------------------------
        for t in range(NSTEPS):
            b, jj = t // JPB, t % JPB
            step_base = state["prefix"]
            ex = scores_exp(b, jj)
            pv_pending.append((b, jj, ex, t))
            state["prefix"] += COST_SCORE
            # one deferred-epilogue DVE atom per step: matches DVE drain rate
            # so the DVE backlog at any later psum alloc stays ~1 item deep
            drain_epi2(1, t)
            drain_mandatory(t)
            budget = (t + 1) * X_PACE
            while True:
                if len(pv_pending) > MAXLAG:
                    if pop_pv(t, force=True):
                        continue
                over = state["prefix"] >= budget
                # min-work floor: even over budget, keep >= MIN_STEP of PE
                # work per step so the PE queue between consecutive scores
                # never thins out (a thin stretch lets the PE race into the
                # scores-psum WAR and stall with ready work queued behind)
                if over and state["prefix"] - step_base >= MIN_STEP:
                    break
                # near-deadline fills ahead of PV pops: pulled ~6 steps
                # early they complete BEFORE epilogue-drain holds, instead
                # of being forced (hard deadline) into a hold where their
                # psum alloc watermark-blocks the PE queue. PV has no
                # deadline and absorbs the displacement.
                held = bool(epi2_queue) or t < fill_hold[0]
                ndl = [
                    g
                    for g in fills
                    if not g.soft
                    and g.deadline <= t + 10
                    and not (held and g.idx == 0 and g.deadline > t + 2)
                    and pickable(g)
                ]
                if ndl:
                    emit_atom(min(ndl, key=lambda g: g.deadline))
                    continue
                if pop_pv(t):
                    continue
                if pop_fill(
                    cont_only=bool(epi2_queue)
                    or t < fill_hold[0]
                    or t < pv_resume[0]
                ):
                    continue
                break

        # ---- tail -------------------------------------------------------
        while pv_pending:
            jj = pv_pending[0][1]
            pop_pv(NSTEPS, force=True)
            if jj == 15:
                # give the pv-bank WAR (epilogue DVE chain) room: interleave
                # outproj work between blocks
                for _ in range(6):
                    if fills:
                        pop_fill()
        while fills:
            if not pop_fill():
                # only blocked psum groups remain; force the first
                emit_atom(sorted(fills, key=lambda g: g.deadline)[0])

    nc.compile()
    return nc


def _get_graph(with_qkv_bias: bool):
    key = ("nc", with_qkv_bias)
    if key not in _CACHE:
        _CACHE[key] = _build_graph(with_qkv_bias)
    return _CACHE[key]


def make_in_maps(x, Wq, bq, Wk, bk, Wv, bv, Wo, with_qkv_bias):
    bf16 = ml_dtypes.bfloat16
    in_maps = []
    for c in range(NCORES):
        b, g = c // GROUPS, c % GROUPS
        hs = slice(g * DH, (g + 1) * DH)
        m = {
            "xt": np.ascontiguousarray(x[b].T.astype(bf16)),
            "wq": np.ascontiguousarray(Wq[hs, :].T.astype(bf16)),
            "wk": np.ascontiguousarray(Wk[hs, :].T.astype(bf16)),
            "wv": np.ascontiguousarray(Wv[hs, :].T.astype(bf16)),
            "wo": np.ascontiguousarray(Wo[:, hs].T.astype(bf16)),
        }
        if with_qkv_bias:
            m["bq"] = np.ascontiguousarray(bq[None, hs].astype(bf16))
            m["bk"] = np.ascontiguousarray(bk[None, hs].astype(bf16))
            m["bv"] = np.ascontiguousarray(bv[None, hs].astype(bf16))
        in_maps.append(m)
    return in_maps


def kernel(x, Wq, bq, Wk, bk, Wv, bv, Wo, bo):
    global last_exec_time_ns, last_results
    from concourse.bass_utils import run_bass_kernel_spmd

    x = np.asarray(x, np.float32)
    Wq = np.asarray(Wq, np.float32)
    Wk = np.asarray(Wk, np.float32)
    Wv = np.asarray(Wv, np.float32)
    Wo = np.asarray(Wo, np.float32)
    bq = np.asarray(bq, np.float32)
    bk = np.asarray(bk, np.float32)
    bv = np.asarray(bv, np.float32)
    bo = np.asarray(bo, np.float32)

    with_qkv_bias = bool(np.any(bq) or np.any(bk) or np.any(bv))
    nc = _get_graph(with_qkv_bias)
    in_maps = make_in_maps(x, Wq, bq, Wk, bk, Wv, bv, Wo, with_qkv_bias)

    trace = os.environ.get("BASS_KERNEL_TRACE", "0") == "1"
    tdir = os.environ.get("BASS_KERNEL_TRACE_DIR") or None
    res = run_bass_kernel_spmd(
        nc, in_maps, list(range(NCORES)), trace=trace, tmpdir=tdir
    )
    last_exec_time_ns = res.exec_time_ns
    last_results = res

    out = np.zeros((B, S, HID), np.float32)
    for c in range(NCORES):
        out[c // GROUPS] += np.asarray(res.results[c]["out"], np.float32)
    out += bo
    return out
